# revision 79
# baseline (speedup 1.0000x reference)
"""Trainium2 Bass kernel for the AttendRNN pair-classifier.

Sharding: pure data-parallel over batch — 8 cores x 8 samples. Each core runs
embedding gather, input projections, the BiGRU recurrence, self-attention with
distance bias, pooling and the final MLP for its 8 sample-pairs. No
cross-core communication; host concatenates the 8 output slices.

Numerics: bf16 storage for all matmul operands (weights, embeddings, xg, h,
attention probabilities), fp32 PSUM accumulation and fp32 elementwise gate
math. Validated vs the fp32 reference at 3e-4..5e-3 max relative error
(varies run to run at hardware level; gate is 2e-2).

Layout notes (per core):
  - Gate/feature dims live on SBUF partitions; batch/time on the free dim.
  - xg[dir]  : [128p, 6 gate-tiles, 16 seq, 256 tok] bf16  (projected inputs)
  - o2T[dir] : [128p, 2 h-halves, 16 seq, 256 tok] bf16    (GRU hidden states,
               written column-by-column during the recurrence; doubles as the
               d-major operand for attention scores)
  - Attention uses S's symmetry: exp(S') tiles serve as both e and e^T, so no
    transpose of the probability matrix is needed; row-normalization is applied
    to e^T via a PE-replicated reciprocal-rowsum vector.
  - Biases: b_ih (all gates) and b_hh (r,z gates) are folded into a constant-1
    input feature (row 300 of the padded W_ih^T). b_hh for the n-gate cannot be
    folded (it sits inside r*(h W^T + b)); it is zero for this model.

The recurrence is chain-latency bound (256 serial steps/direction); the
per-step path is MMs -> sigmoid -> scan1 -> tanh -> scan2 -> next MMs, where
scan1/scan2 are tensor_tensor_scan instructions used as 3-input fused
multiply-adds over interleaved (even,odd) element pairs (even slot resets the
scan state to one operand, odd slot produces a + b*c). The two directions'
chains interleave on the engines at a half-step offset. Half of the attention
phase's o2 transposes are issued on the idle sync DMA queue once the
recurrence passes halfway (fwd cols 0..127 / bwd cols 128..255 are final).
"""

import sys

sys.path.insert(0, "/opt/trn_rl_repo")

import numpy as np
import ml_dtypes

from concourse import bass, mybir
from concourse import bacc
from concourse import tile
from concourse.bass_utils import run_bass_kernel_spmd

BF16NP = ml_dtypes.bfloat16
F32 = mybir.dt.float32
BF = mybir.dt.bfloat16
I32 = mybir.dt.int32

N = 256          # doc length
V = 300          # embed dim
VP = 384         # padded embed dim (3 x 128; col 300 = const-1 bias feature)
H = 256          # GRU hidden
G = 3 * H        # gates
FCD = 512
B = 64
SIGMA = 0.95
VOCAB = 50000
NCORES = 8
BL = B // NCORES          # samples per core
SEQ = 2 * BL              # sequences per direction per core (samples x docs)
NINST = 2 * BL            # attention instances per core (samples x docs)
NTOK = SEQ * N            # gathered tokens per core
TCH = 512                 # token chunk for the input projection
NCH = NTOK // TCH

_CACHE = {}


def _build_program():
    nc = bacc.Bacc(None, target_bir_lowering=False)

    # ---- DRAM I/O ----------------------------------------------------------
    idx_d = nc.dram_tensor("idx", [128, NTOK // 128], I32, kind="ExternalInput")
    embed_d = nc.dram_tensor("embed", [VOCAB, V], BF, kind="ExternalInput")
    wih_d = nc.dram_tensor("wih", [128, 2 * 3 * G], BF, kind="ExternalInput")
    whh_d = nc.dram_tensor("whh", [128, 2 * 2 * G], BF, kind="ExternalInput")
    dist_d = nc.dram_tensor("dist", [128, 2 * N], F32, kind="ExternalInput")
    fc1w_d = nc.dram_tensor("fc1w", [128, 16 * FCD], BF, kind="ExternalInput")
    fc1b_d = nc.dram_tensor("fc1b", [BL, FCD], F32, kind="ExternalInput")
    fc2w_d = nc.dram_tensor("fc2w", [BL, FCD], F32, kind="ExternalInput")
    fc2b_d = nc.dram_tensor("fc2b", [BL, 1], F32, kind="ExternalInput")
    ident_d = nc.dram_tensor("ident", [128, 128], F32, kind="ExternalInput")
    out_d = nc.dram_tensor("out", [BL, 1], F32, kind="ExternalOutput")

    TT = mybir.AluOpType
    AF = mybir.ActivationFunctionType

    with tile.TileContext(nc) as tc:
        with (
            tc.tile_pool(name="const", bufs=1) as cp,
            tc.tile_pool(name="big", bufs=1) as bigp,
        ):
            idx_sb = cp.tile([128, NTOK // 128], I32, tag="idx")
            wih_sb = cp.tile([128, 2 * 3 * G], BF, tag="wih")
            whh_sb = cp.tile([128, 2 * 2 * G], BF, tag="whh")
            dist_sb = cp.tile([128, 2 * N], F32, tag="dist")
            fc1w_sb = cp.tile([128, 16 * FCD], BF, tag="fc1w")
            fc1b_sb = cp.tile([BL, FCD], F32, tag="fc1b")
            fc2w_sb = cp.tile([BL, FCD], F32, tag="fc2w")
            fc2b_sb = cp.tile([BL, 1], F32, tag="fc2b")
            ident_sb = cp.tile([128, 128], F32, tag="ident")
            ident_bf = cp.tile([128, 128], BF, tag="identbf")
            ones_sb = cp.tile([1, 128], F32, tag="ones")

            for dst, src in [(idx_sb, idx_d), (wih_sb, wih_d), (whh_sb, whh_d),
                             (dist_sb, dist_d), (fc1w_sb, fc1w_d),
                             (fc1b_sb, fc1b_d), (fc2w_sb, fc2w_d),
                             (fc2b_sb, fc2b_d), (ident_sb, ident_d)]:
                nc.sync.dma_start(dst[:], src[:])
            nc.vector.memset(ones_sb[:], 1.0)
            ones_bf = cp.tile([1, 128], BF, tag="onesbf")
            nc.vector.memset(ones_bf[:], 1.0)
            nc.vector.tensor_copy(ident_bf[:], ident_sb[:])

            wih_v = wih_sb[:].rearrange("p (d k g) -> p d k g", d=2, k=3)
            whh_v = whh_sb[:].rearrange("p (d k g) -> p d k g", d=2, k=2)
            dist_v = dist_sb[:].rearrange("p (n m) -> p n m", n=2)
            fc1w_v = fc1w_sb[:].rearrange("p (k f) -> p k f", k=16)

            # persistent activations
            xg_t = [bigp.tile([128, 6 * SEQ * N], BF, name=f"xg{d}", tag=f"xg{d}") for d in (0, 1)]
            xg_v = [t[:].rearrange("p (m i t) -> p m i t", m=6, i=SEQ) for t in xg_t]
            o2_t = [bigp.tile([128, 2 * SEQ * N], BF, name=f"o2{d}", tag=f"o2{d}") for d in (0, 1)]
            o2_v = [t[:].rearrange("p (k i t) -> p k i t", k=2, i=SEQ) for t in o2_t]
            o8_sb = bigp.tile([128, 2 * 2 * 4 * BL], F32, tag="o8")
            o8_v = o8_sb[:].rearrange("p (c q f s) -> p c q f s", c=2, q=2, f=4)
            # early-transposed halves of o2m (fwd block0 / bwd block1), filled
            # on the idle sync queue once the recurrence passes halfway
            o2me = bigp.tile([128, NINST * 2 * 256], BF, tag="o2me")
            o2me_v = o2me[:].rearrange("p (i k n) -> p i k n", i=NINST, k=2)


            # ---- Phase 1+2: gather/projection overlapped with BiGRU --------
            # p1: chunks are t-blocks of 32 timesteps x all 16 seqs, ordered
            # from both ends (0,7,1,6,...). Chunk pair k = blocks (k, 7-k)
            # unblocks recurrence steps [32k, 32k+32) for both directions, so
            # chunk emission is interleaved with step emission (engine queues
            # are FIFO in emission order).
            # p2 chain-latency design: the per-step serial path is
            #   hg matmuls -> sigmoid(r,z) -> scan1 -> tanh -> scan2 -> next MMs
            # where scan1/scan2 are tensor_tensor_scan ops used as 3-input
            # FMAs over interleaved (even,odd) element pairs:
            #   even t=2j: state = (0 * state) + c[j]          (reset to c)
            #   odd       : state = (b[j] * c[j]) + a[j]       (a + b*c)
            # scan1: npre = xn + r*hn   (c=hn from PSUM even slots, a=xn
            #        injected to PSUM odd slots by an identity matmul)
            # scan2: h   = zh + (1-z)*n (c=n from tanh, a=z*h_prev)
            # z-products (1-z, z*h_prev) run during the tanh window.
            # Chunk pair k = t-blocks (k, 7-k): unblocks steps [32k, 32k+32)
            # for both directions. Pair 0 is emitted as a prologue; pair k+1's
            # instructions are woven between step emissions of window k so its
            # queue entries arrive with dependencies already resolved
            # (gather/transpose ticks in the first half-window, matmul/copy
            # ticks in the second half).
            BORDER = [0, 7, 1, 6, 2, 5, 3, 4]
            with (
                tc.spectator_scope("p12"),
                tc.tile_pool(name="graw", bufs=1) as rawp,
                tc.tile_pool(name="gtr", bufs=3) as etp,
                tc.tile_pool(name="xps", bufs=2, space="PSUM") as xps,
                tc.tile_pool(name="hgrz", bufs=2, space="PSUM") as hrzpool,
                tc.tile_pool(name="hgn", bufs=2, space="PSUM") as hnpool,
                tc.tile_pool(name="pair0", bufs=3) as pp0,
                tc.tile_pool(name="pair1", bufs=3) as pp1,
                tc.tile_pool(name="gfix", bufs=1) as gf,
            ):
                NRAW = 3
                raws = [rawp.tile([128, VP], BF, name=f"raw{j}", tag=f"raw{j}") for j in range(NRAW)]
                for r in raws:
                    nc.vector.memset(r[:, V:VP], 0.0)
                    nc.vector.memset(r[:, V:V + 1], 1.0)
                et_store = {}

                def gt_ticks(ch, overlap):
                    et = etp.tile([128, 3 * TCH], BF, tag="embT")
                    et_store[ch] = et[:].rearrange("p (k t) -> p k t", k=3)
                    etv = et_store[ch]
                    for j in range(TCH // 128):
                        i = ch * (TCH // 128) + j
                        r = raws[i % NRAW]
                        nc.gpsimd.indirect_dma_start(
                            out=r[:, 0:V], out_offset=None,
                            in_=embed_d[:, :],
                            in_offset=bass.IndirectOffsetOnAxis(
                                ap=idx_sb[:, i:i + 1], axis=0),
                        )
                        yield
                        for kt in range(3):
                            dst = etv[:, kt, j * 128:(j + 1) * 128]
                            src = r[:, kt * 128:(kt + 1) * 128]
                            # overlap keeps scalar free for the chain ACTs
                            teng = (nc.sync if overlap or (i + kt) % 2 == 0
                                    else nc.scalar)
                            teng.dma_start_transpose(dst, src)
                        yield

                def mm_ticks(ch, overlap):
                    blk = BORDER[ch]
                    etv = et_store[ch]
                    for d in (0, 1):
                        for mt in range(6):
                            ps = xps.tile([128, TCH], F32, tag="xgps")
                            for kt in range(3):
                                nc.tensor.matmul(
                                    ps[:], lhsT=wih_v[:, d, kt, mt * 128:(mt + 1) * 128],
                                    rhs=etv[:, kt, :],
                                    start=(kt == 0), stop=(kt == 2))
                            dst = xg_v[d][:, mt, :, 32 * blk:32 * blk + 32]
                            src = ps[:].rearrange("p (i t) -> p i t", i=SEQ)
                            if overlap:
                                nc.vector.tensor_copy(dst, src)
                            else:
                                nc.any.tensor_copy(dst, src)
                            yield

                def rr(ga, gb):
                    gens = [ga, gb]
                    while gens:
                        for g in list(gens):
                            try:
                                yield next(g)
                            except StopIteration:
                                gens.remove(g)

                # prologue: pair 0. Transposes ride the (idle) PE alongside
                # the two HWDGE queues; projections are emitted in two
                # 16-timestep slices so the recurrence starts after slice 0,
                # with slice 1 landing during the first steps' runway.
                et4 = {}
                for ch in (0, 1):
                    et = etp.tile([128, 3 * TCH], BF, tag="embT")
                    et_store[ch] = et[:].rearrange("p (k t) -> p k t", k=3)
                    et4[ch] = et[:].rearrange("p (k i t) -> p k i t",
                                              k=3, i=SEQ)
                    for j in range(TCH // 128):
                        i = ch * (TCH // 128) + j
                        r = raws[i % NRAW]
                        nc.gpsimd.indirect_dma_start(
                            out=r[:, 0:V], out_offset=None,
                            in_=embed_d[:, :],
                            in_offset=bass.IndirectOffsetOnAxis(
                                ap=idx_sb[:, i:i + 1], axis=0),
                        )
                        for kt in range(3):
                            dst = et_store[ch][:, kt, j * 128:(j + 1) * 128]
                            src = r[:, kt * 128:(kt + 1) * 128]
                            if kt == 0 and i % 2 == 0:
                                nc.sync.dma_start_transpose(dst, src)
                            elif kt == 0:
                                nc.scalar.dma_start_transpose(dst, src)
                            else:
                                pt = xps.tile([128, 128], BF, tag="p1t")
                                nc.tensor.transpose(pt[:], src, ident_bf[:])
                                nc.vector.tensor_copy(dst, pt[:])
                for lo, w in ((0, 16), (16, 16)):
                    for ch in (0, 1):
                        blk = BORDER[ch]
                        for d in (0, 1):
                            for mt in range(6):
                                ps = xps.tile([128, SEQ * w], F32, tag="xgps")
                                for kt in range(3):
                                    nc.tensor.matmul(
                                        ps[:],
                                        lhsT=wih_v[:, d, kt, mt * 128:(mt + 1) * 128],
                                        rhs=et4[ch][:, kt, :, lo:lo + w],
                                        start=(kt == 0), stop=(kt == 2))
                                dst = xg_v[d][:, mt, :,
                                              32 * blk + lo:32 * blk + lo + w]
                                nc.any.tensor_copy(
                                    dst, ps[:].rearrange("p (i t) -> p i t", i=SEQ))

                win_gt = {}
                win_mm = {}
                hrzp = [hrzpool, hrzpool]
                hnp = [hnpool, hnpool]
                pairp = [pp0, pp1]
                # dedicated per-direction scratch (evens of srz/s2d0 stay 0)
                srz_t, s1o_t, s2d0_t, s2d1_t = [], [], [], []
                for d in (0, 1):
                    srz = gf.tile([128, 4 * SEQ * 2], F32, tag=f"srzs{d}")
                    s1o = gf.tile([128, 2 * SEQ * 2], F32, tag=f"s1o{d}")
                    s2d0 = gf.tile([128, 2 * SEQ * 2], BF, tag=f"s2d0{d}")
                    s2d1 = gf.tile([128, 2 * SEQ * 2], BF, tag=f"s2d1{d}")
                    nc.vector.memset(srz[:], 0.0)
                    nc.vector.memset(s2d0[:], 0.0)
                    srz_t.append(srz)
                    s1o_t.append(s1o)
                    s2d0_t.append(s2d0)
                    s2d1_t.append(s2d1)
                srz_v = [t[:].rearrange("p (m i v) -> p m i v", m=4, i=SEQ)
                         for t in srz_t]
                s1o_v = [t[:].rearrange("p (m i v) -> p m i v", m=2, i=SEQ)
                         for t in s1o_t]
                s2d0_v = [t[:].rearrange("p (m i v) -> p m i v", m=2, i=SEQ)
                          for t in s2d0_t]
                s2d1_v = [t[:].rearrange("p (m i v) -> p m i v", m=2, i=SEQ)
                          for t in s2d1_t]
                pair_prev = [None, None]
                for t in range(N):
                    if t == 129:
                        # halfway: fwd cols 0..127 and bwd cols 128..255 are
                        # final for every instance -> pre-transpose those o2m
                        # halves on the (idle) sync queue
                        for i3 in range(NINST):
                            for ft in range(4):
                                d3, kt3 = divmod(ft, 2)
                                nt3 = d3  # fwd -> nt0 block, bwd -> nt1 block
                                src = o2_v[d3][:, kt3, i3,
                                               nt3 * 128:(nt3 + 1) * 128]
                                col = (ft if d3 == 0 else ft - 2) * 128
                                nc.sync.dma_start_transpose(
                                    o2me_v[:, i3, nt3, col:col + 128], src)
                    for d in (0, 1):
                        xcol = t if d == 0 else N - 1 - t
                        wcol = xcol
                        xg = xg_v[d]
                        o2 = o2_v[d]
                        srzv = srz_v[d]
                        s1ov = s1o_v[d]
                        s2d0v = s2d0_v[d]
                        s2d1v = s2d1_v[d]
                        pair = pairp[d].tile([128, 2 * SEQ * 2], BF,
                                             tag=f"pair{d}")
                        pairv = pair[:].rearrange("p (m i v) -> p m i v",
                                                  m=2, i=SEQ)
                        if t == 0:
                            # h=0: gates from xg only; zh odd slots <- 0
                            nc.scalar.activation(srzv[:, :, :, 1],
                                                 xg[:, 0:4, :, xcol],
                                                 AF.Sigmoid)
                            nc.scalar.activation(s2d1v[:, :, :, 0],
                                                 xg[:, 4:6, :, xcol], AF.Tanh)
                            nc.vector.memset(s2d1v[:, :, :, 1], 0.0)
                        else:
                            hrz_t = hrzp[d].tile([128, 4 * SEQ], F32,
                                                 tag="hrz")
                            hrz = hrz_t[:]
                            hrzv = hrz.rearrange("p (m i) -> p m i", m=4)
                            hgn_t = hnp[d].tile([128, 2 * SEQ * 2], F32,
                                                tag="hgn")
                            hgn = hgn_t[:]
                            hgnv = hgn.rearrange("p (m i v) -> p m i v",
                                                 m=2, i=SEQ)
                            hprev = pair_prev[d]
                            # r,z: xg inject + weight sweep into PSUM
                            nc.tensor.matmul(
                                hrzv[:, :, :], lhsT=ident_bf[:],
                                rhs=xg[:, 0:4, :, xcol], start=True,
                                stop=False)
                            for mt in range(4):
                                nc.tensor.matmul(
                                    hrzv[:, mt, :],
                                    lhsT=whh_v[:, d, 0, mt * 128:(mt + 1) * 128],
                                    rhs=hprev[:, 0, :, 1], start=False,
                                    stop=False)
                                nc.tensor.matmul(
                                    hrzv[:, mt, :],
                                    lhsT=whh_v[:, d, 1, mt * 128:(mt + 1) * 128],
                                    rhs=hprev[:, 1, :, 1], start=False,
                                    stop=True)
                            # n-gate: hn -> even slots, xn -> odd slots
                            for mt in (0, 1):
                                nc.tensor.matmul(
                                    hgnv[:, mt, :, 1], lhsT=ident_bf[:],
                                    rhs=xg[:, 4 + mt, :, xcol], start=True,
                                    stop=True)
                                nc.tensor.matmul(
                                    hgnv[:, mt, :, 0],
                                    lhsT=whh_v[:, d, 0,
                                               (4 + mt) * 128:(5 + mt) * 128],
                                    rhs=hprev[:, 0, :, 1], start=True,
                                    stop=False)
                                nc.tensor.matmul(
                                    hgnv[:, mt, :, 0],
                                    lhsT=whh_v[:, d, 1,
                                               (4 + mt) * 128:(5 + mt) * 128],
                                    rhs=hprev[:, 1, :, 1], start=False,
                                    stop=True)
                            # sigmoid(r,z) -> odd slots of srz scratch
                            nc.scalar.activation(srzv[:, :, :, 1],
                                                 hrz, AF.Sigmoid)
                            # scan1: odd slots = xn + r*hn
                            nc.vector.tensor_tensor_scan(
                                s1o_t[d][:], srz_t[d][:, 0:4 * SEQ],
                                hgn, 0.0, op0=TT.mult, op1=TT.add)
                            # off-path z-products (during tanh window)
                            nc.gpsimd.tensor_tensor(
                                s2d1v[:, :, :, 1], srzv[:, 2:4, :, 1],
                                hprev[:, :, :, 1], op=TT.mult)
                            nc.scalar.activation(s2d1v[:, :, :, 0],
                                                 s1ov[:, :, :, 1], AF.Tanh)
                        nc.vector.tensor_scalar(
                            s2d0v[:, :, :, 1], srzv[:, 2:4, :, 1],
                            -1.0, 1.0, op0=TT.mult, op1=TT.add)
                        # scan2: odd slots = z*h_prev + (1-z)*n
                        nc.vector.tensor_tensor_scan(
                            pair[:], s2d0_t[d][:], s2d1_t[d][:], 0.0,
                            op0=TT.mult, op1=TT.add)
                        # persist h column for attention (off critical path)
                        nc.gpsimd.tensor_copy(o2[:, :, :, wcol],
                                              pairv[:, :, :, 1])
                        pair_prev[d] = pairv
                    # weave the next chunk pair's instructions in behind this
                    # step's chain ops, paced so deps are resolved on arrival
                    k = t // 32
                    if k <= 2:
                        pos = t % 32
                        if pos == 0:
                            win_gt[k] = rr(gt_ticks(2 * k + 2, True),
                                           gt_ticks(2 * k + 3, True))
                            win_mm[k] = rr(mm_ticks(2 * k + 2, True),
                                           mm_ticks(2 * k + 3, True))
                        if pos < 16:
                            next(win_gt[k], None)
                        else:
                            for _ in range(2 if pos % 2 == 0 else 1):
                                next(win_mm[k], None)
                    elif t == 96:
                        for g in list(win_gt.values()) + list(win_mm.values()):
                            for _ in g:
                                pass

            # ---- Phase 3: attention + pooling ------------------------------
            with (
                tc.spectator_scope("p3_attn"),
                tc.tile_pool(name="o2m", bufs=4) as o2mp,
                tc.tile_pool(name="sps", bufs=3, space="PSUM") as sps,
                tc.tile_pool(name="wps", bufs=1, space="PSUM") as wps,
                tc.tile_pool(name="o5ps", bufs=2, space="PSUM") as o5ps,
                tc.tile_pool(name="o5n", bufs=2) as o5np,
                tc.tile_pool(name="att", bufs=4) as ap,
            ):
                for i in range(NINST):
                    doc = i % 2
                    s = i // 2
                    # m-major copy of o2 for the o5 matmul; half the tiles
                    # were pre-transposed during the recurrence (o2me), the
                    # other half go on the two HWDGE queues here
                    o2ml = o2mp.tile([128, 2 * 256], BF, tag="o2m")
                    o2mlv = o2ml[:].rearrange("p (k dd) -> p k dd", k=2)
                    late_pos = 0
                    for ft in range(4):
                        d, kt = divmod(ft, 2)
                        for nt in range(2):
                            if d == nt:
                                continue  # early tile, already in o2me
                            src = o2_v[d][:, kt, i, nt * 128:(nt + 1) * 128]
                            dst = o2mlv[:, nt, (ft % 2) * 128:(ft % 2) * 128 + 128]
                            teng = nc.sync if late_pos % 2 == 0 else nc.scalar
                            teng.dma_start_transpose(dst, src)
                            late_pos += 1

                    def o2m_tile(km, dc):
                        col = (dc % 2) * 128
                        if (dc // 2) == km:
                            return o2me_v[:, i, km, col:col + 128]
                        return o2mlv[:, km, col:col + 128]
                    # scores S' = o2 @ o2^T - dist  (both [256, 256], symmetric)
                    sp = sps.tile([128, 2 * N], F32, tag="sps")
                    spv = sp[:].rearrange("p (n m) -> p n m", n=2)
                    for nt in range(2):
                        for ft in range(4):
                            d, kt = divmod(ft, 2)
                            nc.tensor.matmul(
                                spv[:, nt, :],
                                lhsT=o2_v[d][:, kt, i, nt * 128:(nt + 1) * 128],
                                rhs=o2_v[d][:, kt, i, :],
                                start=(ft == 0), stop=(ft == 3))
                    nc.vector.tensor_tensor(sp[:], sp[:], dist_sb[:], op=TT.subtract)
                    # e = exp(S'), rowsum via accumulate output; symmetric => e == e^T
                    e_sb = ap.tile([128, 2 * N], BF, tag="esb")
                    ev = e_sb[:].rearrange("p (n m) -> p n m", n=2)
                    rs = ap.tile([128, 2], F32, tag="rs")
                    for nt in range(2):
                        nc.scalar.activation(ev[:, nt, :], spv[:, nt, :], AF.Exp,
                                             accum_out=rs[:, nt:nt + 1])
                    rcp = ap.tile([128, 2], F32, tag="rcp")
                    nc.vector.reciprocal(rcp[:], rs[:])
                    # replicate 1/rowsum across partitions: transpose + ones-
                    # outer. Runs concurrently with the o5 matmuls below —
                    # normalization is applied after the matmul, so rcp/wrep
                    # are off the per-instance critical path.
                    wpx = wps.tile([128, 2 * 128 + N], F32, tag="wpx")
                    wrow = ap.tile([1, 2 * 128], F32, tag="wrow")
                    for nt in range(2):
                        nc.tensor.transpose(wpx[0:1, nt * 128:(nt + 1) * 128],
                                            rcp[:, nt:nt + 1], ident_sb[:])
                        nc.vector.tensor_copy(wrow[0:1, nt * 128:(nt + 1) * 128],
                                              wpx[0:1, nt * 128:(nt + 1) * 128])
                    wrep = wpx[:, 2 * 128:]
                    nc.tensor.matmul(wrep, lhsT=ones_sb[:, :],
                                     rhs=wrow[0:1, :], start=True, stop=True)
                    wrep_sb = ap.tile([128, N], F32, tag="wrsb")
                    nc.scalar.activation(wrep_sb[:], wrep, AF.Copy)
                    # o5u^T[d, n] = sum_m o2m[m, d] * e^T[m, n]  (unnormalized;
                    # e tiles double as e^T by symmetry)
                    o5 = o5ps.tile([128, 4 * N], F32, tag="o5")
                    o5v = o5[:].rearrange("p (f n) -> p f n", f=4)
                    for dc in range(4):
                        for km in range(2):
                            nc.tensor.matmul(
                                o5v[:, dc, :],
                                lhsT=o2m_tile(km, dc),
                                rhs=ev[:, km, :],
                                start=(km == 0), stop=(km == 1))
                    # normalize + mean(sum) pool fused per d-tile; then max
                    o5n = o5np.tile([128, 4 * N], BF, tag="o5n")
                    o5nv = o5n[:].rearrange("p (f n) -> p f n", f=4)
                    for dc in range(4):
                        nc.vector.scalar_tensor_tensor(
                            o5nv[:, dc, :], o5v[:, dc, :], 1.0, wrep_sb[:],
                            op0=TT.mult, op1=TT.mult,
                            accum_out=o8_v[:, doc, 0, dc, s:s + 1])
                    nc.vector.tensor_reduce(o8_v[:, doc, 1, :, s],
                                            o5nv[:, :, :], axis=mybir.AxisListType.X,
                                            op=TT.max)

            # ---- Phase 4: final MLP ---------------------------------------
            with (
                tc.spectator_scope("p4_fc"),
                tc.tile_pool(name="fc", bufs=1) as fp,
                tc.tile_pool(name="fcps", bufs=1, space="PSUM") as fps,
            ):
                dsub = fp.tile([128, 2 * 4 * BL], F32, tag="dsub")
                zall = fp.tile([128, 2 * 2 * 4 * BL], BF, tag="zall")
                zv = zall[:].rearrange("p (z q f s) -> p z q f s", z=2, q=2, f=4)
                dv = dsub[:].rearrange("p (q f s) -> p q f s", q=2, f=4)
                nc.vector.tensor_tensor(dsub[:], o8_v[:, 0, :, :, :],
                                        o8_v[:, 1, :, :, :], op=TT.subtract)
                nc.scalar.activation(zv[:, 0, :, :, :], dv[:, :, :, :], AF.Abs)
                nc.vector.tensor_tensor(zv[:, 1, :, :, :], o8_v[:, 0, :, :, :],
                                        o8_v[:, 1, :, :, :], op=TT.mult)
                h1p = fps.tile([BL, FCD], F32, tag="h1p")
                zk = zall[:].rearrange("p (k s) -> p k s", k=16)
                for k in range(16):
                    nc.tensor.matmul(h1p[:], lhsT=zk[:, k, :], rhs=fc1w_v[:, k, :],
                                     start=(k == 0), stop=(k == 15))
                h1 = fp.tile([BL, FCD], F32, tag="h1")
                nc.vector.tensor_tensor(h1[:], h1p[:], fc1b_sb[:], op=TT.add)
                h1r = fp.tile([BL, FCD], F32, tag="h1r")
                nc.scalar.activation(h1r[:], h1[:], AF.Relu)
                prod = fp.tile([BL, FCD], F32, tag="prod")
                nc.vector.tensor_tensor(prod[:], h1r[:], fc2w_sb[:], op=TT.mult)
                acc = fp.tile([BL, 1], F32, tag="acc")
                nc.vector.tensor_reduce(acc[:], prod[:], axis=mybir.AxisListType.X,
                                        op=TT.add)
                res = fp.tile([BL, 1], F32, tag="res")
                nc.scalar.activation(res[:], acc[:], AF.Sigmoid, bias=fc2b_sb[:, 0:1])
                nc.sync.dma_start(out_d[:], res[:])

    nc.compile()
    return nc


def _prep_shared(embed, W_ih_f, W_hh_f, b_ih_f, b_hh_f, W_ih_b, W_hh_b,
                 b_ih_b, b_hh_b, fc1_w, fc1_b, fc2_w, fc2_b):
    embed_bf = np.ascontiguousarray(np.asarray(embed, np.float32)).astype(BF16NP)

    def pack_wih(W, b_ih, b_hh):
        Wt = np.zeros((VP, G), np.float32)
        Wt[:V] = np.asarray(W, np.float32).T
        bias = np.asarray(b_ih, np.float32).copy()
        bias[:2 * H] += np.asarray(b_hh, np.float32)[:2 * H]
        Wt[V] = bias
        return Wt.reshape(3, 128, G).transpose(1, 0, 2)

    wih = np.stack([pack_wih(W_ih_f, b_ih_f, b_hh_f),
                    pack_wih(W_ih_b, b_ih_b, b_hh_b)], axis=1)  # [128, 2, 3, G]
    wih = np.ascontiguousarray(wih.reshape(128, -1)).astype(BF16NP)

    def pack_whh(W):
        Wt = np.asarray(W, np.float32).T.reshape(2, 128, G).transpose(1, 0, 2)
        return Wt

    whh = np.stack([pack_whh(W_hh_f), pack_whh(W_hh_b)], axis=1)
    whh = np.ascontiguousarray(whh.reshape(128, -1)).astype(BF16NP)

    i = np.arange(N, dtype=np.float32)
    dist = ((i[:, None] - i[None, :]) ** 2) / SIGMA
    dist = np.ascontiguousarray(dist.reshape(2, 128, N).transpose(1, 0, 2)
                                .reshape(128, -1)).astype(np.float32)

    fc1wT = np.asarray(fc1_w, np.float32).T.copy()      # [2048, 512]
    fc1wT[0:512] *= 1.0 / N                             # |a-b| mean block
    fc1wT[1024:1536] *= 1.0 / (N * N)                   # a*b mean block
    fc1w = np.ascontiguousarray(fc1wT.reshape(16, 128, FCD).transpose(1, 0, 2)
                                .reshape(128, -1)).astype(BF16NP)

    fc1b = np.broadcast_to(np.asarray(fc1_b, np.float32), (BL, FCD)).copy()
    fc2w = np.broadcast_to(np.asarray(fc2_w, np.float32).reshape(1, FCD),
                           (BL, FCD)).copy()
    fc2b = np.full((BL, 1), np.float32(np.asarray(fc2_b).reshape(-1)[0]))
    ident = np.eye(128, dtype=np.float32)
    return dict(embed=embed_bf, wih=wih, whh=whh, dist=dist, fc1w=fc1w,
                fc1b=fc1b, fc2w=fc2w, fc2b=fc2b, ident=ident)


def kernel(x, embed, W_ih_f, W_hh_f, b_ih_f, b_hh_f, W_ih_b, W_hh_b,
           b_ih_b, b_hh_b, fc1_w, fc1_b, fc2_w, fc2_b, _profile=None):
    shared = _prep_shared(embed, W_ih_f, W_hh_f, b_ih_f, b_hh_f, W_ih_b,
                          W_hh_b, b_ih_b, b_hh_b, fc1_w, fc1_b, fc2_w, fc2_b)
    x = np.asarray(x).astype(np.int32)  # [B, 2, N]
    border = [0, 7, 1, 6, 2, 5, 3, 4]
    in_maps = []
    for c in range(NCORES):
        xs = x[c * BL:(c + 1) * BL].reshape(SEQ, N)       # (i=s*2+doc, t)
        # chunk ch = t-block border[ch]: tokens ordered (i, t-within-block)
        tok = xs.reshape(SEQ, NCH, N // NCH)[:, border, :]
        tok = np.ascontiguousarray(tok.transpose(1, 0, 2)).reshape(-1)
        idx = np.ascontiguousarray(tok.reshape(NTOK // 128, 128).T)
        in_maps.append({"idx": idx, **shared})

    if "nc" not in _CACHE:
        _CACHE["nc"] = _build_program()
    nc = _CACHE["nc"]

    kw = {}
    if _profile is not None:
        kw = dict(trace=True, tmpdir=_profile)
    res = run_bass_kernel_spmd(nc, in_maps, list(range(NCORES)), **kw)
    out = np.concatenate([res.results[c]["out"].reshape(-1)
                          for c in range(NCORES)])
    if _profile is not None:
        return out.astype(np.float32), res
    return out.astype(np.float32)



# revision 80
# speedup vs baseline: 1.1926x; 1.1926x over previous
"""Trainium2 Bass kernel for the AttendRNN pair-classifier.

Sharding: pure data-parallel over batch — 8 cores x 8 samples. Each core runs
embedding gather, input projections, the BiGRU recurrence, self-attention with
distance bias, pooling and the final MLP for its 8 sample-pairs. No
cross-core communication; host concatenates the 8 output slices.

Numerics: bf16 storage for all matmul operands (weights, embeddings, xg, h,
attention probabilities), fp32 PSUM accumulation and fp32 elementwise gate
math. Validated vs the fp32 reference at 3e-4..5e-3 max relative error
(varies run to run at hardware level; gate is 2e-2).

Layout notes (per core):
  - Gate/feature dims live on SBUF partitions; batch/time on the free dim.
  - xg[dir]  : [128p, 6 gate-tiles, 16 seq, 256 tok] bf16  (projected inputs)
  - o2T[dir] : [128p, 2 h-halves, 16 seq, 256 tok] bf16    (GRU hidden states,
               written column-by-column during the recurrence; doubles as the
               d-major operand for attention scores)
  - Attention uses S's symmetry: exp(S') tiles serve as both e and e^T, so no
    transpose of the probability matrix is needed; row-normalization is applied
    to e^T via a PE-replicated reciprocal-rowsum vector.
  - Biases: b_ih (all gates) and b_hh (r,z gates) are folded into a constant-1
    input feature (row 300 of the padded W_ih^T). b_hh for the n-gate cannot be
    folded (it sits inside r*(h W^T + b)); it is zero for this model.

The recurrence is chain-latency bound (256 serial steps/direction); the
per-step path is MMs -> sigmoid -> scan1 -> tanh -> scan2 -> next MMs, where
scan1/scan2 are tensor_tensor_scan instructions used as 3-input fused
multiply-adds over interleaved (even,odd) element pairs (even slot resets the
scan state to one operand, odd slot produces a + b*c). The two directions'
chains interleave on the engines at a half-step offset. Half of the attention
phase's o2 transposes are issued on the idle sync DMA queue once the
recurrence passes halfway (fwd cols 0..127 / bwd cols 128..255 are final).
"""

import sys

sys.path.insert(0, "/opt/trn_rl_repo")

import numpy as np
import ml_dtypes

from concourse import bass, mybir
from concourse import bacc
from concourse import tile
from concourse.bass_utils import run_bass_kernel_spmd

BF16NP = ml_dtypes.bfloat16
F32 = mybir.dt.float32
BF = mybir.dt.bfloat16
I32 = mybir.dt.int32

N = 256          # doc length
V = 300          # embed dim
VP = 384         # padded embed dim (3 x 128; col 300 = const-1 bias feature)
H = 256          # GRU hidden
G = 3 * H        # gates
FCD = 512
B = 64
SIGMA = 0.95
VOCAB = 50000
NCORES = 8
BL = B // NCORES          # samples per core
SEQ = 2 * BL              # sequences per direction per core (samples x docs)
NINST = 2 * BL            # attention instances per core (samples x docs)
NTOK = SEQ * N            # gathered tokens per core
TCH = 512                 # token chunk for the input projection
NCH = NTOK // TCH

_CACHE = {}


def _build_program():
    nc = bacc.Bacc(None, target_bir_lowering=False)

    # ---- DRAM I/O ----------------------------------------------------------
    idx_d = nc.dram_tensor("idx", [128, NTOK // 128], I32, kind="ExternalInput")
    embed_d = nc.dram_tensor("embed", [VOCAB, V], BF, kind="ExternalInput")
    wih_d = nc.dram_tensor("wih", [128, 2 * 3 * G], BF, kind="ExternalInput")
    whh_d = nc.dram_tensor("whh", [128, 2 * 2 * G], BF, kind="ExternalInput")
    dist_d = nc.dram_tensor("dist", [128, 2 * N], F32, kind="ExternalInput")
    fc1w_d = nc.dram_tensor("fc1w", [128, 16 * FCD], BF, kind="ExternalInput")
    fc1b_d = nc.dram_tensor("fc1b", [BL, FCD], F32, kind="ExternalInput")
    fc2w_d = nc.dram_tensor("fc2w", [BL, FCD], F32, kind="ExternalInput")
    fc2b_d = nc.dram_tensor("fc2b", [BL, 1], F32, kind="ExternalInput")
    ident_d = nc.dram_tensor("ident", [128, 128], F32, kind="ExternalInput")
    out_d = nc.dram_tensor("out", [BL, 1], F32, kind="ExternalOutput")

    TT = mybir.AluOpType
    AF = mybir.ActivationFunctionType

    with tile.TileContext(nc) as tc:
        with (
            tc.tile_pool(name="const", bufs=1) as cp,
            tc.tile_pool(name="big", bufs=1) as bigp,
        ):
            idx_sb = cp.tile([128, NTOK // 128], I32, tag="idx")
            wih_sb = cp.tile([128, 2 * 3 * G], BF, tag="wih")
            whh_sb = cp.tile([128, 2 * 2 * G], BF, tag="whh")
            dist_sb = cp.tile([128, 2 * N], F32, tag="dist")
            fc1w_sb = cp.tile([128, 16 * FCD], BF, tag="fc1w")
            fc1b_sb = cp.tile([BL, FCD], F32, tag="fc1b")
            fc2w_sb = cp.tile([BL, FCD], F32, tag="fc2w")
            fc2b_sb = cp.tile([BL, 1], F32, tag="fc2b")
            ident_sb = cp.tile([128, 128], F32, tag="ident")
            ident_bf = cp.tile([128, 128], BF, tag="identbf")
            ones_sb = cp.tile([1, 128], F32, tag="ones")

            for dst, src in [(idx_sb, idx_d), (wih_sb, wih_d), (whh_sb, whh_d),
                             (dist_sb, dist_d), (fc1w_sb, fc1w_d),
                             (fc1b_sb, fc1b_d), (fc2w_sb, fc2w_d),
                             (fc2b_sb, fc2b_d), (ident_sb, ident_d)]:
                nc.sync.dma_start(dst[:], src[:])
            nc.vector.memset(ones_sb[:], 1.0)
            ones_bf = cp.tile([1, 128], BF, tag="onesbf")
            nc.vector.memset(ones_bf[:], 1.0)
            nc.vector.tensor_copy(ident_bf[:], ident_sb[:])

            wih_v = wih_sb[:].rearrange("p (d k g) -> p d k g", d=2, k=3)
            whh_v = whh_sb[:].rearrange("p (d k g) -> p d k g", d=2, k=2)
            dist_v = dist_sb[:].rearrange("p (n m) -> p n m", n=2)
            fc1w_v = fc1w_sb[:].rearrange("p (k f) -> p k f", k=16)

            # persistent activations
            xg_t = [bigp.tile([128, 6 * SEQ * N], BF, name=f"xg{d}", tag=f"xg{d}") for d in (0, 1)]
            xg_v = [t[:].rearrange("p (m i t) -> p m i t", m=6, i=SEQ) for t in xg_t]
            o2_t = [bigp.tile([128, 2 * SEQ * N], BF, name=f"o2{d}", tag=f"o2{d}") for d in (0, 1)]
            o2_v = [t[:].rearrange("p (k i t) -> p k i t", k=2, i=SEQ) for t in o2_t]
            o8_sb = bigp.tile([128, 2 * 2 * 4 * BL], F32, tag="o8")
            o8_v = o8_sb[:].rearrange("p (c q f s) -> p c q f s", c=2, q=2, f=4)
            # early-transposed halves of o2m (fwd block0 / bwd block1), filled
            # on the idle sync queue once the recurrence passes halfway
            o2me = bigp.tile([128, NINST * 2 * 256], BF, tag="o2me")
            o2me_v = o2me[:].rearrange("p (i k n) -> p i k n", i=NINST, k=2)


            # ---- Phase 1+2: gather/projection overlapped with BiGRU --------
            # p1: chunks are t-blocks of 32 timesteps x all 16 seqs, ordered
            # from both ends (0,7,1,6,...). Chunk pair k = blocks (k, 7-k)
            # unblocks recurrence steps [32k, 32k+32) for both directions, so
            # chunk emission is interleaved with step emission (engine queues
            # are FIFO in emission order).
            # p2 chain-latency design: the per-step serial path is
            #   hg matmuls -> sigmoid(r,z) -> scan1 -> tanh -> scan2 -> next MMs
            # where scan1/scan2 are tensor_tensor_scan ops used as 3-input
            # FMAs over interleaved (even,odd) element pairs:
            #   even t=2j: state = (0 * state) + c[j]          (reset to c)
            #   odd       : state = (b[j] * c[j]) + a[j]       (a + b*c)
            # scan1: npre = xn + r*hn   (c=hn from PSUM even slots, a=xn
            #        injected to PSUM odd slots by an identity matmul)
            # scan2: h   = zh + (1-z)*n (c=n from tanh, a=z*h_prev)
            # z-products (1-z, z*h_prev) run during the tanh window.
            # Chunk pair k = t-blocks (k, 7-k): unblocks steps [32k, 32k+32)
            # for both directions. Pair 0 is emitted as a prologue; pair k+1's
            # instructions are woven between step emissions of window k so its
            # queue entries arrive with dependencies already resolved
            # (gather/transpose ticks in the first half-window, matmul/copy
            # ticks in the second half).
            BORDER = [0, 7, 1, 6, 2, 5, 3, 4]
            with (
                tc.spectator_scope("p12"),
                tc.tile_pool(name="graw", bufs=1) as rawp,
                tc.tile_pool(name="gtr", bufs=3) as etp,
                tc.tile_pool(name="xps", bufs=2, space="PSUM") as xps,
                tc.tile_pool(name="hgrz", bufs=2, space="PSUM") as hrzpool,
                tc.tile_pool(name="hgn", bufs=2, space="PSUM") as hnpool,
                tc.tile_pool(name="pair0", bufs=3) as pp0,
                tc.tile_pool(name="pair1", bufs=3) as pp1,
                tc.tile_pool(name="gfix", bufs=1) as gf,
            ):
                NRAW = 3
                raws = [rawp.tile([128, VP], BF, name=f"raw{j}", tag=f"raw{j}") for j in range(NRAW)]
                for r in raws:
                    nc.vector.memset(r[:, V:VP], 0.0)
                    nc.vector.memset(r[:, V:V + 1], 1.0)
                et_store = {}

                def gt_ticks(ch, overlap):
                    et = etp.tile([128, 3 * TCH], BF, tag="embT")
                    et_store[ch] = et[:].rearrange("p (k t) -> p k t", k=3)
                    etv = et_store[ch]
                    for j in range(TCH // 128):
                        i = ch * (TCH // 128) + j
                        r = raws[i % NRAW]
                        nc.gpsimd.indirect_dma_start(
                            out=r[:, 0:V], out_offset=None,
                            in_=embed_d[:, :],
                            in_offset=bass.IndirectOffsetOnAxis(
                                ap=idx_sb[:, i:i + 1], axis=0),
                        )
                        yield
                        for kt in range(3):
                            dst = etv[:, kt, j * 128:(j + 1) * 128]
                            src = r[:, kt * 128:(kt + 1) * 128]
                            # overlap keeps scalar free for the chain ACTs
                            teng = (nc.sync if overlap or (i + kt) % 2 == 0
                                    else nc.scalar)
                            teng.dma_start_transpose(dst, src)
                        yield

                def mm_ticks(ch, overlap):
                    blk = BORDER[ch]
                    etv = et_store[ch]
                    for d in (0, 1):
                        for mt in range(6):
                            ps = xps.tile([128, TCH], F32, tag="xgps")
                            for kt in range(3):
                                nc.tensor.matmul(
                                    ps[:], lhsT=wih_v[:, d, kt, mt * 128:(mt + 1) * 128],
                                    rhs=etv[:, kt, :],
                                    start=(kt == 0), stop=(kt == 2))
                            dst = xg_v[d][:, mt, :, 32 * blk:32 * blk + 32]
                            src = ps[:].rearrange("p (i t) -> p i t", i=SEQ)
                            if overlap:
                                nc.vector.tensor_copy(dst, src)
                            else:
                                nc.any.tensor_copy(dst, src)
                            yield

                def rr(ga, gb):
                    gens = [ga, gb]
                    while gens:
                        for g in list(gens):
                            try:
                                yield next(g)
                            except StopIteration:
                                gens.remove(g)

                # prologue: pair 0. Transposes ride the (idle) PE alongside
                # the two HWDGE queues; projections are emitted in two
                # 16-timestep slices so the recurrence starts after slice 0,
                # with slice 1 landing during the first steps' runway.
                et4 = {}
                for ch in (0, 1):
                    et = etp.tile([128, 3 * TCH], BF, tag="embT")
                    et_store[ch] = et[:].rearrange("p (k t) -> p k t", k=3)
                    et4[ch] = et[:].rearrange("p (k i t) -> p k i t",
                                              k=3, i=SEQ)
                    for j in range(TCH // 128):
                        i = ch * (TCH // 128) + j
                        r = raws[i % NRAW]
                        nc.gpsimd.indirect_dma_start(
                            out=r[:, 0:V], out_offset=None,
                            in_=embed_d[:, :],
                            in_offset=bass.IndirectOffsetOnAxis(
                                ap=idx_sb[:, i:i + 1], axis=0),
                        )
                        for kt in range(3):
                            dst = et_store[ch][:, kt, j * 128:(j + 1) * 128]
                            src = r[:, kt * 128:(kt + 1) * 128]
                            if kt == 0 and i % 2 == 0:
                                nc.sync.dma_start_transpose(dst, src)
                            elif kt == 0:
                                nc.scalar.dma_start_transpose(dst, src)
                            else:
                                pt = xps.tile([128, 128], BF, tag="p1t")
                                nc.tensor.transpose(pt[:], src, ident_bf[:])
                                nc.vector.tensor_copy(dst, pt[:])
                for lo, w in ((0, 16), (16, 16)):
                    for ch in (0, 1):
                        blk = BORDER[ch]
                        for d in (0, 1):
                            for mt in range(6):
                                ps = xps.tile([128, SEQ * w], F32, tag="xgps")
                                for kt in range(3):
                                    nc.tensor.matmul(
                                        ps[:],
                                        lhsT=wih_v[:, d, kt, mt * 128:(mt + 1) * 128],
                                        rhs=et4[ch][:, kt, :, lo:lo + w],
                                        start=(kt == 0), stop=(kt == 2))
                                dst = xg_v[d][:, mt, :,
                                              32 * blk + lo:32 * blk + lo + w]
                                nc.any.tensor_copy(
                                    dst, ps[:].rearrange("p (i t) -> p i t", i=SEQ))

                win_gt = {}
                win_mm = {}
                hrzp = [hrzpool, hrzpool]
                hnp = [hnpool, hnpool]
                pairp = [pp0, pp1]
                # dedicated per-direction scratch (evens of srz/s2d0 stay 0)
                srz_t, s1o_t, s2d0_t, s2d1_t = [], [], [], []
                for d in (0, 1):
                    srz = gf.tile([128, 4 * SEQ * 2], F32, tag=f"srzs{d}")
                    s1o = gf.tile([128, 2 * SEQ * 2], F32, tag=f"s1o{d}")
                    s2d0 = gf.tile([128, 2 * SEQ * 2], BF, tag=f"s2d0{d}")
                    s2d1 = gf.tile([128, 2 * SEQ * 2], BF, tag=f"s2d1{d}")
                    nc.vector.memset(srz[:], 0.0)
                    nc.vector.memset(s2d0[:], 0.0)
                    srz_t.append(srz)
                    s1o_t.append(s1o)
                    s2d0_t.append(s2d0)
                    s2d1_t.append(s2d1)
                srz_v = [t[:].rearrange("p (m i v) -> p m i v", m=4, i=SEQ)
                         for t in srz_t]
                s1o_v = [t[:].rearrange("p (m i v) -> p m i v", m=2, i=SEQ)
                         for t in s1o_t]
                s2d0_v = [t[:].rearrange("p (m i v) -> p m i v", m=2, i=SEQ)
                          for t in s2d0_t]
                s2d1_v = [t[:].rearrange("p (m i v) -> p m i v", m=2, i=SEQ)
                          for t in s2d1_t]
                pair_prev = [None, None]
                for t in range(N):
                    if t == 129:
                        # halfway: fwd cols 0..127 and bwd cols 128..255 are
                        # final for every instance -> pre-transpose those o2m
                        # halves on the (idle) sync queue
                        for i3 in range(NINST):
                            for ft in range(4):
                                d3, kt3 = divmod(ft, 2)
                                nt3 = d3  # fwd -> nt0 block, bwd -> nt1 block
                                src = o2_v[d3][:, kt3, i3,
                                               nt3 * 128:(nt3 + 1) * 128]
                                col = (ft if d3 == 0 else ft - 2) * 128
                                nc.sync.dma_start_transpose(
                                    o2me_v[:, i3, nt3, col:col + 128], src)
                    for d in (0, 1):
                        xcol = t if d == 0 else N - 1 - t
                        wcol = xcol
                        xg = xg_v[d]
                        o2 = o2_v[d]
                        srzv = srz_v[d]
                        s1ov = s1o_v[d]
                        s2d0v = s2d0_v[d]
                        s2d1v = s2d1_v[d]
                        pair = pairp[d].tile([128, 2 * SEQ * 2], BF,
                                             tag=f"pair{d}")
                        pairv = pair[:].rearrange("p (m i v) -> p m i v",
                                                  m=2, i=SEQ)
                        if t == 0:
                            # h=0: gates from xg only; zh odd slots <- 0
                            nc.scalar.activation(srzv[:, :, :, 1],
                                                 xg[:, 0:4, :, xcol],
                                                 AF.Sigmoid)
                            nc.scalar.activation(s2d1v[:, :, :, 0],
                                                 xg[:, 4:6, :, xcol], AF.Tanh)
                            nc.vector.memset(s2d1v[:, :, :, 1], 0.0)
                        else:
                            hrz_t = hrzp[d].tile([128, 4 * SEQ], F32,
                                                 tag="hrz")
                            hrz = hrz_t[:]
                            hrzv = hrz.rearrange("p (m i) -> p m i", m=4)
                            hgn_t = hnp[d].tile([128, 2 * SEQ * 2], F32,
                                                tag="hgn")
                            hgn = hgn_t[:]
                            hgnv = hgn.rearrange("p (m i v) -> p m i v",
                                                 m=2, i=SEQ)
                            hprev = pair_prev[d]
                            # r,z: xg inject + weight sweep into PSUM
                            nc.tensor.matmul(
                                hrzv[:, :, :], lhsT=ident_bf[:],
                                rhs=xg[:, 0:4, :, xcol], start=True,
                                stop=False)
                            for mt in range(4):
                                nc.tensor.matmul(
                                    hrzv[:, mt, :],
                                    lhsT=whh_v[:, d, 0, mt * 128:(mt + 1) * 128],
                                    rhs=hprev[:, 0, :, 1], start=False,
                                    stop=False)
                                nc.tensor.matmul(
                                    hrzv[:, mt, :],
                                    lhsT=whh_v[:, d, 1, mt * 128:(mt + 1) * 128],
                                    rhs=hprev[:, 1, :, 1], start=False,
                                    stop=True)
                            # n-gate: hn -> even slots, xn -> odd slots
                            for mt in (0, 1):
                                nc.tensor.matmul(
                                    hgnv[:, mt, :, 1], lhsT=ident_bf[:],
                                    rhs=xg[:, 4 + mt, :, xcol], start=True,
                                    stop=True)
                                nc.tensor.matmul(
                                    hgnv[:, mt, :, 0],
                                    lhsT=whh_v[:, d, 0,
                                               (4 + mt) * 128:(5 + mt) * 128],
                                    rhs=hprev[:, 0, :, 1], start=True,
                                    stop=False)
                                nc.tensor.matmul(
                                    hgnv[:, mt, :, 0],
                                    lhsT=whh_v[:, d, 1,
                                               (4 + mt) * 128:(5 + mt) * 128],
                                    rhs=hprev[:, 1, :, 1], start=False,
                                    stop=True)
                            # sigmoid(r,z) -> odd slots of srz scratch
                            nc.scalar.activation(srzv[:, :, :, 1],
                                                 hrz, AF.Sigmoid)
                            # scan1: odd slots = xn + r*hn
                            nc.vector.tensor_tensor_scan(
                                s1o_t[d][:], srz_t[d][:, 0:4 * SEQ],
                                hgn, 0.0, op0=TT.mult, op1=TT.add)
                            # off-path z-products (during tanh window)
                            nc.gpsimd.tensor_tensor(
                                s2d1v[:, :, :, 1], srzv[:, 2:4, :, 1],
                                hprev[:, :, :, 1], op=TT.mult)
                            nc.scalar.activation(s2d1v[:, :, :, 0],
                                                 s1ov[:, :, :, 1], AF.Tanh)
                        nc.gpsimd.tensor_scalar(
                            s2d0v[:, :, :, 1], srzv[:, 2:4, :, 1],
                            -1.0, 1.0, op0=TT.mult, op1=TT.add)
                        # scan2: odd slots = z*h_prev + (1-z)*n
                        nc.vector.tensor_tensor_scan(
                            pair[:], s2d0_t[d][:], s2d1_t[d][:], 0.0,
                            op0=TT.mult, op1=TT.add)
                        # persist h column for attention (off critical path)
                        nc.gpsimd.tensor_copy(o2[:, :, :, wcol],
                                              pairv[:, :, :, 1])
                        pair_prev[d] = pairv
                    # weave the next chunk pair's instructions in behind this
                    # step's chain ops, paced so deps are resolved on arrival
                    k = t // 32
                    if k <= 2:
                        pos = t % 32
                        if pos == 0:
                            win_gt[k] = rr(gt_ticks(2 * k + 2, True),
                                           gt_ticks(2 * k + 3, True))
                            win_mm[k] = rr(mm_ticks(2 * k + 2, True),
                                           mm_ticks(2 * k + 3, True))
                        if pos < 16:
                            next(win_gt[k], None)
                        else:
                            for _ in range(2 if pos % 2 == 0 else 1):
                                next(win_mm[k], None)
                    elif t == 96:
                        for g in list(win_gt.values()) + list(win_mm.values()):
                            for _ in g:
                                pass

            # ---- Phase 3: attention + pooling ------------------------------
            with (
                tc.spectator_scope("p3_attn"),
                tc.tile_pool(name="o2m", bufs=4) as o2mp,
                tc.tile_pool(name="sps", bufs=3, space="PSUM") as sps,
                tc.tile_pool(name="wps", bufs=1, space="PSUM") as wps,
                tc.tile_pool(name="o5ps", bufs=2, space="PSUM") as o5ps,
                tc.tile_pool(name="o5n", bufs=2) as o5np,
                tc.tile_pool(name="att", bufs=4) as ap,
            ):
                for i in range(NINST):
                    doc = i % 2
                    s = i // 2
                    # m-major copy of o2 for the o5 matmul; half the tiles
                    # were pre-transposed during the recurrence (o2me), the
                    # other half go on the two HWDGE queues here
                    o2ml = o2mp.tile([128, 2 * 256], BF, tag="o2m")
                    o2mlv = o2ml[:].rearrange("p (k dd) -> p k dd", k=2)
                    late_pos = 0
                    for ft in range(4):
                        d, kt = divmod(ft, 2)
                        for nt in range(2):
                            if d == nt:
                                continue  # early tile, already in o2me
                            src = o2_v[d][:, kt, i, nt * 128:(nt + 1) * 128]
                            dst = o2mlv[:, nt, (ft % 2) * 128:(ft % 2) * 128 + 128]
                            teng = nc.sync if late_pos % 2 == 0 else nc.scalar
                            teng.dma_start_transpose(dst, src)
                            late_pos += 1

                    def o2m_tile(km, dc):
                        col = (dc % 2) * 128
                        if (dc // 2) == km:
                            return o2me_v[:, i, km, col:col + 128]
                        return o2mlv[:, km, col:col + 128]
                    # scores S' = o2 @ o2^T - dist  (both [256, 256], symmetric)
                    sp = sps.tile([128, 2 * N], F32, tag="sps")
                    spv = sp[:].rearrange("p (n m) -> p n m", n=2)
                    for nt in range(2):
                        for ft in range(4):
                            d, kt = divmod(ft, 2)
                            nc.tensor.matmul(
                                spv[:, nt, :],
                                lhsT=o2_v[d][:, kt, i, nt * 128:(nt + 1) * 128],
                                rhs=o2_v[d][:, kt, i, :],
                                start=(ft == 0), stop=(ft == 3))
                    nc.vector.tensor_tensor(sp[:], sp[:], dist_sb[:], op=TT.subtract)
                    # e = exp(S'), rowsum via accumulate output; symmetric => e == e^T
                    e_sb = ap.tile([128, 2 * N], BF, tag="esb")
                    ev = e_sb[:].rearrange("p (n m) -> p n m", n=2)
                    rs = ap.tile([128, 2], F32, tag="rs")
                    for nt in range(2):
                        nc.scalar.activation(ev[:, nt, :], spv[:, nt, :], AF.Exp,
                                             accum_out=rs[:, nt:nt + 1])
                    rcp = ap.tile([128, 2], F32, tag="rcp")
                    nc.vector.reciprocal(rcp[:], rs[:])
                    # replicate 1/rowsum across partitions: transpose + ones-
                    # outer. Runs concurrently with the o5 matmuls below —
                    # normalization is applied after the matmul, so rcp/wrep
                    # are off the per-instance critical path.
                    wpx = wps.tile([128, 2 * 128 + N], F32, tag="wpx")
                    wrow = ap.tile([1, 2 * 128], F32, tag="wrow")
                    for nt in range(2):
                        nc.tensor.transpose(wpx[0:1, nt * 128:(nt + 1) * 128],
                                            rcp[:, nt:nt + 1], ident_sb[:])
                        nc.vector.tensor_copy(wrow[0:1, nt * 128:(nt + 1) * 128],
                                              wpx[0:1, nt * 128:(nt + 1) * 128])
                    wrep = wpx[:, 2 * 128:]
                    nc.tensor.matmul(wrep, lhsT=ones_sb[:, :],
                                     rhs=wrow[0:1, :], start=True, stop=True)
                    wrep_sb = ap.tile([128, N], F32, tag="wrsb")
                    nc.scalar.activation(wrep_sb[:], wrep, AF.Copy)
                    # o5u^T[d, n] = sum_m o2m[m, d] * e^T[m, n]  (unnormalized;
                    # e tiles double as e^T by symmetry)
                    o5 = o5ps.tile([128, 4 * N], F32, tag="o5")
                    o5v = o5[:].rearrange("p (f n) -> p f n", f=4)
                    for dc in range(4):
                        for km in range(2):
                            nc.tensor.matmul(
                                o5v[:, dc, :],
                                lhsT=o2m_tile(km, dc),
                                rhs=ev[:, km, :],
                                start=(km == 0), stop=(km == 1))
                    # normalize + mean(sum) pool fused per d-tile; then max
                    o5n = o5np.tile([128, 4 * N], BF, tag="o5n")
                    o5nv = o5n[:].rearrange("p (f n) -> p f n", f=4)
                    for dc in range(4):
                        nc.vector.scalar_tensor_tensor(
                            o5nv[:, dc, :], o5v[:, dc, :], 1.0, wrep_sb[:],
                            op0=TT.mult, op1=TT.mult,
                            accum_out=o8_v[:, doc, 0, dc, s:s + 1])
                    nc.vector.tensor_reduce(o8_v[:, doc, 1, :, s],
                                            o5nv[:, :, :], axis=mybir.AxisListType.X,
                                            op=TT.max)

            # ---- Phase 4: final MLP ---------------------------------------
            with (
                tc.spectator_scope("p4_fc"),
                tc.tile_pool(name="fc", bufs=1) as fp,
                tc.tile_pool(name="fcps", bufs=1, space="PSUM") as fps,
            ):
                dsub = fp.tile([128, 2 * 4 * BL], F32, tag="dsub")
                zall = fp.tile([128, 2 * 2 * 4 * BL], BF, tag="zall")
                zv = zall[:].rearrange("p (z q f s) -> p z q f s", z=2, q=2, f=4)
                dv = dsub[:].rearrange("p (q f s) -> p q f s", q=2, f=4)
                nc.vector.tensor_tensor(dsub[:], o8_v[:, 0, :, :, :],
                                        o8_v[:, 1, :, :, :], op=TT.subtract)
                nc.scalar.activation(zv[:, 0, :, :, :], dv[:, :, :, :], AF.Abs)
                nc.vector.tensor_tensor(zv[:, 1, :, :, :], o8_v[:, 0, :, :, :],
                                        o8_v[:, 1, :, :, :], op=TT.mult)
                h1p = fps.tile([BL, FCD], F32, tag="h1p")
                zk = zall[:].rearrange("p (k s) -> p k s", k=16)
                for k in range(16):
                    nc.tensor.matmul(h1p[:], lhsT=zk[:, k, :], rhs=fc1w_v[:, k, :],
                                     start=(k == 0), stop=(k == 15))
                h1 = fp.tile([BL, FCD], F32, tag="h1")
                nc.vector.tensor_tensor(h1[:], h1p[:], fc1b_sb[:], op=TT.add)
                h1r = fp.tile([BL, FCD], F32, tag="h1r")
                nc.scalar.activation(h1r[:], h1[:], AF.Relu)
                prod = fp.tile([BL, FCD], F32, tag="prod")
                nc.vector.tensor_tensor(prod[:], h1r[:], fc2w_sb[:], op=TT.mult)
                acc = fp.tile([BL, 1], F32, tag="acc")
                nc.vector.tensor_reduce(acc[:], prod[:], axis=mybir.AxisListType.X,
                                        op=TT.add)
                res = fp.tile([BL, 1], F32, tag="res")
                nc.scalar.activation(res[:], acc[:], AF.Sigmoid, bias=fc2b_sb[:, 0:1])
                nc.sync.dma_start(out_d[:], res[:])

    nc.compile()
    return nc


def _prep_shared(embed, W_ih_f, W_hh_f, b_ih_f, b_hh_f, W_ih_b, W_hh_b,
                 b_ih_b, b_hh_b, fc1_w, fc1_b, fc2_w, fc2_b):
    embed_bf = np.ascontiguousarray(np.asarray(embed, np.float32)).astype(BF16NP)

    def pack_wih(W, b_ih, b_hh):
        Wt = np.zeros((VP, G), np.float32)
        Wt[:V] = np.asarray(W, np.float32).T
        bias = np.asarray(b_ih, np.float32).copy()
        bias[:2 * H] += np.asarray(b_hh, np.float32)[:2 * H]
        Wt[V] = bias
        return Wt.reshape(3, 128, G).transpose(1, 0, 2)

    wih = np.stack([pack_wih(W_ih_f, b_ih_f, b_hh_f),
                    pack_wih(W_ih_b, b_ih_b, b_hh_b)], axis=1)  # [128, 2, 3, G]
    wih = np.ascontiguousarray(wih.reshape(128, -1)).astype(BF16NP)

    def pack_whh(W):
        Wt = np.asarray(W, np.float32).T.reshape(2, 128, G).transpose(1, 0, 2)
        return Wt

    whh = np.stack([pack_whh(W_hh_f), pack_whh(W_hh_b)], axis=1)
    whh = np.ascontiguousarray(whh.reshape(128, -1)).astype(BF16NP)

    i = np.arange(N, dtype=np.float32)
    dist = ((i[:, None] - i[None, :]) ** 2) / SIGMA
    dist = np.ascontiguousarray(dist.reshape(2, 128, N).transpose(1, 0, 2)
                                .reshape(128, -1)).astype(np.float32)

    fc1wT = np.asarray(fc1_w, np.float32).T.copy()      # [2048, 512]
    fc1wT[0:512] *= 1.0 / N                             # |a-b| mean block
    fc1wT[1024:1536] *= 1.0 / (N * N)                   # a*b mean block
    fc1w = np.ascontiguousarray(fc1wT.reshape(16, 128, FCD).transpose(1, 0, 2)
                                .reshape(128, -1)).astype(BF16NP)

    fc1b = np.broadcast_to(np.asarray(fc1_b, np.float32), (BL, FCD)).copy()
    fc2w = np.broadcast_to(np.asarray(fc2_w, np.float32).reshape(1, FCD),
                           (BL, FCD)).copy()
    fc2b = np.full((BL, 1), np.float32(np.asarray(fc2_b).reshape(-1)[0]))
    ident = np.eye(128, dtype=np.float32)
    return dict(embed=embed_bf, wih=wih, whh=whh, dist=dist, fc1w=fc1w,
                fc1b=fc1b, fc2w=fc2w, fc2b=fc2b, ident=ident)


def kernel(x, embed, W_ih_f, W_hh_f, b_ih_f, b_hh_f, W_ih_b, W_hh_b,
           b_ih_b, b_hh_b, fc1_w, fc1_b, fc2_w, fc2_b, _profile=None):
    shared = _prep_shared(embed, W_ih_f, W_hh_f, b_ih_f, b_hh_f, W_ih_b,
                          W_hh_b, b_ih_b, b_hh_b, fc1_w, fc1_b, fc2_w, fc2_b)
    x = np.asarray(x).astype(np.int32)  # [B, 2, N]
    border = [0, 7, 1, 6, 2, 5, 3, 4]
    in_maps = []
    for c in range(NCORES):
        xs = x[c * BL:(c + 1) * BL].reshape(SEQ, N)       # (i=s*2+doc, t)
        # chunk ch = t-block border[ch]: tokens ordered (i, t-within-block)
        tok = xs.reshape(SEQ, NCH, N // NCH)[:, border, :]
        tok = np.ascontiguousarray(tok.transpose(1, 0, 2)).reshape(-1)
        idx = np.ascontiguousarray(tok.reshape(NTOK // 128, 128).T)
        in_maps.append({"idx": idx, **shared})

    if "nc" not in _CACHE:
        _CACHE["nc"] = _build_program()
    nc = _CACHE["nc"]

    kw = {}
    if _profile is not None:
        kw = dict(trace=True, tmpdir=_profile)
    res = run_bass_kernel_spmd(nc, in_maps, list(range(NCORES)), **kw)
    out = np.concatenate([res.results[c]["out"].reshape(-1)
                          for c in range(NCORES)])
    if _profile is not None:
        return out.astype(np.float32), res
    return out.astype(np.float32)

